# revision 53
# baseline (speedup 1.0000x reference)
"""Multi-head causal attention (B=2, T=2048, D=1024, H=16, HD=64) on 8 TRN2
NeuronCores.

Sharding v2: head-pair x both batches. Core c handles heads {2c, 2c+1} for
BOTH batch elements. Wq/Wk/Wv are split column-wise (128 cols per core), Wo
row-wise; each core produces a full [T, D] partial output per batch (its 2
heads' contribution), which the host sums across all 8 cores per batch.

Vs v1 (batch x head-group): identical per-core FLOPs, but the projection work
for batch 1 and the output projection for batch 0 become *late* dense matmul
work that interleaves into the attention windows of heads 2-4, keeping the PE
array active enough that HAM stays at K=8/8 (v1 ran the whole second half at
K=4/8 half clock because only K=64/M=65 half-array matmuls remained).

v2.1: adds a ~3.4us HAM warm-up block (back-to-back matmuls on a zero tile)
so the PE clock is already at K=8/8 when the first real matmul's DMA deps
land (~8us) instead of ramping at ~18us. Two warm-up PSUM allocations keep
the psProj slot-rotation parity unchanged for every downstream "proj" unit
(a single allocation shifts the rotation and perturbs the delicate w2/w3
boundary, re-throttling HAM mid-kernel).

Window schedule (16 k-tile slots each):
  pre : QT(b0) c0-3 + KT(b0) c0
  w0  : ST/CT(b0,h0) + V(b0) tiles + KT(b0) c1-3
  w1  : ST/CT(b0,h1) + QT(b1) c0-3 + KT(b1) c0 + V(b1) tt0-7
  w2  : ST/CT(b1,h0) + V(b1) tt8-15 + KT(b1) c1-3 + out(b0) tt0-7
  w3  : ST/CT(b1,h1) + out(b0) tt8-15 + out(b1) tt0-3
  tail: out(b1) tt4-15 rotating over the freed psST/psO/psCT banks
        (3-unit pipeline; no mid-stream pool close)

v2.3 tweaks: the 8 b0 first-half x loads alternate sync/scalar queues
(scalar is idle until the first exp ~13us; two queues halve the dispatch +
transfer serialization on the first QKT unit's critical 2.25MB), and tail
out DMAs alternate sync/scalar. Measured mean over 4 profile runs:
178.4us (v2.1) -> 177.4us. Keep-warm dummy matmuls in the tail were tried
and did NOT hold the HAM clock-gate at K=8/8 (cold segments unchanged).

v2.4: per-head ET tags for kt>=10 remove the ~2.5us window-boundary stalls
where the next window's first ST matmuls waited (WAR) on the previous
window's gpsimd mask-mul readers of the reused ET buffer. Steady-state
175.7-176.8us. (ohp bufs must stay 4: 6 gave wrong results.)

CT dribble is relaxed (group j stops at slot 4j+7) in w0/w1 and front-loaded
(stop at slot 4j+4) in w2/w3 so the late norm chains don't pile up at the
window end. The (b1,h1) norm-chain bounce DMAs ride the gpsimd SWDGE queue
(idle by then); bulk x/out transfers stay on sync — tiny DMAs behind them
would eat ~10us of completion-semaphore-pool latency per leg. Out DMAs all
on sync; out-PSUM evacuation casts run at 1x (~1.2us per [128,1024] tile),
split DVE/ACT in the tail where ACT is idle.

NOTE on rejected restructures (measured, see session traces): pairing the
two heads' K=64 score matmuls as adjacent row-tiles ((0,0)/(64,0)) DOES run
2x on the hardware (218ns/pair microbenchmark, mb_pair.py), but any schedule
that compresses ST into fewer windows concentrates both heads' exps there
and the single ACT engine (58us of exp total) becomes the window pacer; the
PE duty cycle drops, the HAM clock-gate re-throttles to K=4/8 for tens of
us, and the kernel nets out slower (195-231us). The v2 layout keeps ACT at
~50% per window, which is what holds K=8/8 through the body.

Per-core kernel mechanics (bf16 operands, fp32 PSUM accumulation) are as in
v1: QT/KT computed transposed [2*64hd, T]; V natural [T, 2*64hd] with a
ones-column per head (stride 66) so P@V also yields softmax row-sums (M=65);
scores transposed ST[k, q], exact-causal, exp'd on ACT (scale=1/8) to bf16 ET;
diagonal-block mask via gpsimd mul; CT accumulates [65, 512] per q-chunk;
reciprocal row-sums broadcast via DRAM bounce, multiplied into packed bf16
CTG [128, T]; out[t, d] = CTG.T @ wo per 128-row tile, fp16 partials."""

import contextlib

import numpy as np

T, D = 2048, 1024
NH, HD = 16, 64
HPC = 2  # heads per core per batch
NB = 2  # batches (both on every core)
NCORES = 8
ND = D // 128  # 8 d-tiles
NT = T // 128  # 16 t/k-tiles
NQ = T // 512  # 4 q-chunks

_NC = None


def _build_nc():
    import concourse.mybir as mybir
    import concourse.tile as tile
    from concourse import bacc
    from concourse.masks import make_upper_triangular

    f32 = mybir.dt.float32
    bf16 = mybir.dt.bfloat16
    fp16 = mybir.dt.float16
    Exp = mybir.ActivationFunctionType.Exp

    nc = bacc.Bacc("TRN2", target_bir_lowering=False, debug=False, num_devices=NCORES)

    xT_d = [nc.dram_tensor(f"xT{b}", [D, T], bf16, kind="ExternalInput").ap() for b in range(NB)]
    # weights pre-rearranged on host to [128, ND*128] (p, dt, c) so the DMA is
    # a single linear blit instead of a strided gather
    wq_d = nc.dram_tensor("wq", [128, ND * 128], bf16, kind="ExternalInput").ap()
    wk_d = nc.dram_tensor("wk", [128, ND * 128], bf16, kind="ExternalInput").ap()
    wv_d = nc.dram_tensor("wv", [128, ND * 128], bf16, kind="ExternalInput").ap()
    wo_d = nc.dram_tensor("wo", [HPC * HD, D], bf16, kind="ExternalInput").ap()
    out_d = [nc.dram_tensor(f"out{b}", [T, D], fp16, kind="ExternalOutput").ap() for b in range(NB)]
    rscr = nc.dram_tensor("rscr", [128, 64], f32).ap()
    rscr2 = nc.dram_tensor("rscr2", [128, 64], f32).ap()

    with tile.TileContext(nc) as tc, contextlib.ExitStack() as ctx:
        pool = lambda **kw: ctx.enter_context(tc.tile_pool(**kw))
        constp = pool(name="const", bufs=1)
        qkp = pool(name="qk", bufs=1)
        vp = pool(name="vpool", bufs=1)
        wop = pool(name="wop", bufs=1)
        etp = pool(name="et", bufs=1)
        stgp = pool(name="stg", bufs=1)
        ctgp = pool(name="ctg", bufs=1)
        normp = pool(name="norm", bufs=2)
        rbp = pool(name="rb", bufs=6)
        # NOTE: bufs=4 is correctness-load-bearing — raising to 6 produced
        # wrong results (rel ~1.0), another rotation-sensitive dependency
        # like the psProj warm-up parity. Do not change without re-verifying.
        ohp = pool(name="oh", bufs=4)
        bctx = contextlib.ExitStack()
        psST = bctx.enter_context(tc.tile_pool(name="psST", bufs=2, space="PSUM"))
        psCT = bctx.enter_context(tc.tile_pool(name="psCT", bufs=2, space="PSUM"))
        actx = contextlib.ExitStack()
        apool = lambda **kw: actx.enter_context(tc.tile_pool(**kw))
        xtp = apool(name="xtr", bufs=1)
        wtp = apool(name="wtiles", bufs=1)
        psProj = apool(name="psProj", bufs=2, space="PSUM")

        mask = constp.tile([128, 128], bf16, name="mask")
        make_upper_triangular(nc, mask[:], val=1.0, diag=True)

        # HAM warm-up: ~3.4us of back-to-back matmuls on a zero tile so the
        # PE clock is at K=8/8 by the time the first real matmul's DMA deps
        # land. Two allocations so the psProj slot rotation keeps its
        # original parity for all downstream "proj" units.
        warm = constp.tile([128, 512], bf16, name="warm")
        nc.vector.memset(warm[:], 0.0)
        wps = [psProj.tile([128, 512], f32, name=f"warmps{i}", tag="proj") for i in range(2)]
        for i in range(8):
            nc.tensor.matmul(wps[i % 2][:], warm[:, 0:128], warm[:], start=True, stop=True)

        # QT/KT per batch: [128 = 2 heads x 64hd, T]
        QT = [qkp.tile([128, T], bf16, name=f"QT{b}") for b in range(NB)]
        KT = [qkp.tile([128, T], bf16, name=f"KT{b}") for b in range(NB)]
        # V natural per batch: [128 t, 66*HPC] with ones-columns
        vsb = [[vp.tile([128, 66 * HPC], bf16, name=f"v{b}_{tt}") for tt in range(NT)] for b in range(NB)]
        wo_sb = wop.tile([128, D], bf16, name="wo_sb")

        # ---------- loads ----------
        wsb = {}
        wtiles = {}

        def load_w(wname, wd):
            wsb[wname] = wtp.tile([128, ND * 128], bf16, name=f"{wname}sb", tag=f"{wname}sb")
            nc.sync.dma_start(wsb[wname][:], wd)
            wtiles[wname] = [wsb[wname][:, 128 * dt : 128 * (dt + 1)] for dt in range(ND)]

        xtr = [
            [xtp.tile([128, T], bf16, name=f"xtr{b}_{dt}", tag=f"xtr{b}_{dt}") for dt in range(ND)]
            for b in range(NB)
        ]
        # b0 x in half-tiles, dt-major per half: the first QKT units (chunks
        # c0/c1 + KT c0) consume only the first halves, pipelining compute at
        # DMA cadence; c2/c3 unblock as the second halves land. Everything on
        # the sync queue: xtr0 gets full HBM bandwidth first, xtr1 streams
        # right behind (done well before window 1 needs it).
        load_w("wq", wq_d)
        for dt in range(ND):
            # first halves alternate sync/scalar: the scalar queue is idle
            # until the first exp (~13us), and two queues halve the
            # dispatch+transfer serialization on the first QKT unit's
            # critical 2.25MB
            [nc.sync, nc.scalar][dt % 2].dma_start(
                xtr[0][dt][:, 0:1024], xT_d[0][128 * dt : 128 * (dt + 1), 0:1024]
            )
        load_w("wk", wk_d)
        for dt in range(ND):
            nc.sync.dma_start(xtr[0][dt][:, 1024:T], xT_d[0][128 * dt : 128 * (dt + 1), 1024:T])
        load_w("wv", wv_d)
        nc.sync.dma_start(wo_sb[:], wo_d)
        for dt in range(ND):
            nc.sync.dma_start(xtr[1][dt][:], xT_d[1][128 * dt : 128 * (dt + 1), :])

        # ---------- emission units ----------
        def emit_qkt_unit(wname, outs, b, c):
            ps = psProj.tile([128, 512], f32, name=f"pj_{wname}{b}_{c}", tag="proj")
            for dt in range(ND):
                nc.tensor.matmul(
                    ps[:],
                    wtiles[wname][dt][:],
                    xtr[b][dt][:, 512 * c : 512 * (c + 1)],
                    start=(dt == 0),
                    stop=(dt == ND - 1),
                )
            nc.vector.tensor_copy(outs[b][:, 512 * c : 512 * (c + 1)], ps[:])

        def emit_v(b, tt):
            ps = psProj.tile([128, 128], f32, name=f"vps{b}_{tt}", tag="proj")
            for dt in range(ND):
                nc.tensor.matmul(
                    ps[:],
                    xtr[b][dt][:, 128 * tt : 128 * (tt + 1)],
                    wtiles["wv"][dt][:],
                    start=(dt == 0),
                    stop=(dt == ND - 1),
                )
            nc.any.memset(vsb[b][tt][:, 64 : 66 * HPC : 66], 1.0)
            for h in range(HPC):
                nc.vector.tensor_copy(
                    vsb[b][tt][:, 66 * h : 66 * h + 64], ps[:, 64 * h : 64 * (h + 1)]
                )

        ets = {}  # (b, h, kt) -> ET tile

        def emit_st(b, h, kt):
            p0 = 64 * h
            w = T - 128 * kt
            # per-head tags for the small late k-tiles: the tag-buffer WAR
            # then reaches back two windows (same h, other batch) instead of
            # one, so the next window's first ST matmuls stop stalling on the
            # previous window's gpsimd mask-mul readers (~2.5us gaps at the
            # w2/w3 boundary). Costs only ~5.4KB of SBUF.
            tg = f"et{kt}_{h}" if kt >= 10 else f"et{kt}"
            et = etp.tile([128, w], bf16, name=f"et_b{b}h{h}_kt{kt}", tag=tg)
            ets[(b, h, kt)] = et
            for sub in range((w + 1023) // 1024):
                sw = min(1024, w - 1024 * sub)
                q0 = 128 * kt + 1024 * sub
                ps = psST.tile([128, sw], f32, name=f"st_b{b}h{h}_k{kt}_s{sub}", tag="st")
                for c in range((sw + 511) // 512):
                    n = min(512, sw - 512 * c)
                    nc.tensor.matmul(
                        ps[:, 512 * c : 512 * c + n],
                        KT[b][p0 : p0 + 64, 128 * kt : 128 * (kt + 1)],
                        QT[b][p0 : p0 + 64, q0 + 512 * c : q0 + 512 * c + n],
                        start=True,
                        stop=True,
                    )
                nc.scalar.activation(
                    et[:, 1024 * sub : 1024 * sub + sw], ps[:, 0:sw], Exp, scale=0.125
                )
            nc.gpsimd.tensor_mul(et[:, 0:128], et[:, 0:128], mask[:])

        stg = {}
        ct_ps = {}

        def emit_ct_mms(b, h, j, kts, first, last):
            if first:
                ct_ps[(b, h, j)] = psCT.tile([65, 512], f32, name=f"ct_b{b}h{h}_j{j}", tag="ct")
            ct = ct_ps[(b, h, j)]
            for kt in kts:
                etoff = 512 * j - 128 * kt
                if etoff >= 0:
                    n, psoff, ecol = 512, 0, etoff
                else:
                    n, psoff, ecol = 512 + etoff, -etoff, 0
                nc.tensor.matmul(
                    ct[0:65, psoff : psoff + n],
                    vsb[b][kt][:, 66 * h : 66 * h + 65],
                    ets[(b, h, kt)][:, ecol : ecol + n],
                    start=(kt == 0),
                    stop=(last and kt == kts[-1]),
                )

        def finish_ct(b, h, j):
            ct = ct_ps[(b, h, j)]
            s = stgp.tile([65, 512], f32, name=f"stg_b{b}h{h}_j{j}")
            stg[(b, h, j)] = s
            nc.vector.tensor_copy(s[:], ct[:])
            idx = 8 * b + 4 * h + j
            dq = nc.gpsimd if (b, h) == (1, 1) else nc.sync
            dq.dma_start(rscr[8 * idx : 8 * idx + 8, :], s[64:65, :])

        CTG = [ctgp.tile([128, T], bf16, name=f"ctg{b}") for b in range(NB)]
        rscr2v = rscr2.rearrange("(r p) c -> r (p c)", p=8)  # [16, 512] view

        def emit_norm(b, h, j):
            # the last window's chains ride the (by then idle) gpsimd SWDGE
            # queue so they don't wait behind bulk out/x transfers on sync
            dq = nc.gpsimd if (b, h) == (1, 1) else nc.sync
            idx = 8 * b + 4 * h + j
            rs_hj = normp.tile([8, 64], f32, name=f"rs{idx}", tag="rs")
            dq.dma_start(rs_hj[:], rscr[8 * idx : 8 * idx + 8, :])
            rc_hj = normp.tile([8, 64], f32, name=f"rc{idx}", tag="rc")
            nc.vector.reciprocal(rc_hj[:], rs_hj[:])
            dq.dma_start(rscr2[8 * idx : 8 * idx + 8, :], rc_hj[:])
            rb = rbp.tile([64, 512], f32, name=f"rb{idx}", tag="rb")
            dq.dma_start(rb[:], rscr2v[idx : idx + 1, :].partition_broadcast(64))
            eng = nc.vector if j in (0, 3) else nc.gpsimd
            eng.tensor_mul(
                CTG[b][64 * h : 64 * h + 64, 512 * j : 512 * (j + 1)],
                stg[(b, h, j)][0:64, :],
                rb[:],
            )

        # ---------- dribble patterns (per window) ----------
        # relaxed: group j stops at slot 4j+7 — smooth PE load, used for the
        # early windows. front: group j stops at slot 4j+4 (earliest the
        # causal ET availability allows) — used for the late windows so their
        # norm chains don't all pile up at the window end and gate the tail.
        drib_relaxed = {sw: [] for sw in range(NT)}
        for j in range(3):
            kts = list(range(4 * j + 4))
            for sl in range(4):
                chunk = kts[(j + 1) * sl : (j + 1) * (sl + 1)]
                drib_relaxed[4 * j + 4 + sl].append((j, chunk, sl == 0, sl == 3))
        for sl in range(1, 15):
            drib_relaxed[sl].append((3, [sl - 1], sl == 1, False))
        drib_relaxed[15].append((3, [14, 15], False, True))

        drib_front = {sw: [] for sw in range(NT)}
        for j in range(3):
            kts = list(range(4 * j + 4))
            for sl in range(4):
                chunk = kts[(j + 1) * sl : (j + 1) * (sl + 1)]
                drib_front[sl + 1 + j * 4].append((j, chunk, sl == 0, sl == 3))
        for sl in range(2, 15):
            drib_front[sl].append((3, [sl - 2], sl == 2, False))
        drib_front[15].append((3, [13, 14, 15], False, True))
        dribbles = [drib_relaxed, drib_relaxed, drib_front, drib_front]

        # out-projection unit (one 128-row t-tile of batch b). The PSUM
        # evacuation cast runs at 1x (~1.2us per tile) — the real cost — so
        # out units are spread across w2 (sharing psProj's "proj" slots), w3
        # (dedicated 2-bank psO pool), and the tail (psST "st" slots, where
        # ACT is idle and takes half the copies).
        psO_holder = {}

        def emit_out(b, tt, scalar_copy=False, wide=False, dma_eng=None, split_dma=False):
            pool_, tag = psO_holder["pool"], psO_holder["tag"]
            oh = ohp.tile([128, D], fp16, name=f"oh{b}_{tt}", tag="oh")
            if wide:
                ps = pool_.tile([128, D], f32, name=f"ops{b}_{tt}", tag=tag)
                for dc in range(2):
                    nc.tensor.matmul(
                        ps[:, 512 * dc : 512 * (dc + 1)],
                        CTG[b][:, 128 * tt : 128 * (tt + 1)],
                        wo_sb[:, 512 * dc : 512 * (dc + 1)],
                        start=True,
                        stop=True,
                    )
                if scalar_copy:
                    nc.scalar.copy(oh[:], ps[:])
                else:
                    nc.vector.tensor_copy(oh[:], ps[:])
            else:
                for dc in range(2):
                    ps = pool_.tile(
                        [128, 512], f32, name=f"ops{b}_{tt}_{dc}", tag=tag
                    )
                    nc.tensor.matmul(
                        ps[:],
                        CTG[b][:, 128 * tt : 128 * (tt + 1)],
                        wo_sb[:, 512 * dc : 512 * (dc + 1)],
                        start=True,
                        stop=True,
                    )
                    if scalar_copy:
                        nc.scalar.copy(oh[:, 512 * dc : 512 * (dc + 1)], ps[:])
                    else:
                        nc.vector.tensor_copy(oh[:, 512 * dc : 512 * (dc + 1)], ps[:])
                    if split_dma:
                        # half-DMA right after each cast so the transfer
                        # overlaps the second half's evacuation
                        (dma_eng or nc.sync).dma_start(
                            out_d[b][128 * tt : 128 * (tt + 1), 512 * dc : 512 * (dc + 1)],
                            oh[:, 512 * dc : 512 * (dc + 1)],
                        )
            if not split_dma:
                (dma_eng or nc.sync).dma_start(out_d[b][128 * tt : 128 * (tt + 1), :], oh[:])
            return oh

        # out-projection slotting: b0 tt0-7 in w2 (slots 8-15, psProj slots),
        # b0 tt8-15 + b1 tt0-3 in w3 (b1 norms for j=0 complete by slot ~10)
        w2_outs = {sw: [] for sw in range(NT)}
        for i in range(8):
            w2_outs[8 + i].append((0, i))
        w3_outs = {sw: [] for sw in range(NT)}
        for i in range(8):
            w3_outs[(i * 16) // 8].append((0, 8 + i))
        for i in range(4):
            w3_outs[12 + i].append((1, i))

        # ---------- schedule ----------
        emit_qkt_unit("wq", QT, 0, 0)
        emit_qkt_unit("wq", QT, 0, 1)
        emit_qkt_unit("wk", KT, 0, 0)
        emit_qkt_unit("wq", QT, 0, 2)
        emit_qkt_unit("wq", QT, 0, 3)

        for win in range(4):
            b, h = win // 2, win % 2
            for sw in range(NT):
                emit_st(b, h, sw)
                if win == 0:
                    emit_v(0, sw)
                    if 1 <= sw <= 3:
                        emit_qkt_unit("wk", KT, 0, sw)
                elif win == 1:
                    if sw % 2 == 0 and sw < 8:
                        emit_qkt_unit("wq", QT, 1, sw // 2)
                    elif sw == 9:
                        emit_qkt_unit("wk", KT, 1, 0)
                    if sw >= 8:
                        emit_v(1, sw - 8)
                elif win == 2:
                    if sw < 8:
                        emit_v(1, sw + 8)
                    if 1 <= sw <= 3:
                        emit_qkt_unit("wk", KT, 1, sw)
                    psO_holder["pool"] = psProj
                    psO_holder["tag"] = "proj"
                    for tt in w2_outs[sw]:
                        emit_out(*tt)
                elif win == 3:
                    if sw == 0:
                        psO_pool = bctx.enter_context(
                            tc.tile_pool(name="psO", bufs=2, space="PSUM")
                        )
                        psO_holder["pool"] = psO_pool
                        psO_holder["tag"] = "ops"
                    for tt in w3_outs[sw]:
                        emit_out(*tt)
                for j, kts_, first, last in dribbles[win][sw]:
                    emit_ct_mms(b, h, j, kts_, first, last)
                    if last:
                        finish_ct(b, h, j)
                        emit_norm(b, h, j)
                if win == 2 and sw == NT - 1:
                    actx.close()

        # tail: remaining b1 out-projection rotating across the three freed
        # PSUM pools — psST "st" ([128,1024] wide), psO "ops" and psCT "ct"
        # ([128,512] pairs) — for 3-unit-deep pipelining (closing pools
        # mid-stream emits a multi-us engine DRAIN, so reuse, don't close).
        # ACT is idle here so it takes half the copies; out DMAs alternate
        # sync/scalar queues.
        rot = [(psST, "st", True), (psO_pool, "ops", False), (psCT, "ct", False)]
        for i, tt in enumerate(range(4, 16)):
            psO_holder["pool"], psO_holder["tag"], wide = rot[i % 3]
            emit_out(
                1,
                tt,
                scalar_copy=(tt % 2 == 0),
                wide=wide,
                dma_eng=[nc.sync, nc.scalar][i % 2],
            )
        bctx.close()

    nc.compile()
    return nc


def _get_nc():
    global _NC
    if _NC is None:
        _NC = _build_nc()
    return _NC


def make_in_maps(x, wq, wk, wv, wo):
    import ml_dtypes

    bf = ml_dtypes.bfloat16

    def rearr(w, cs):
        # [D, 128] -> [128, ND*128] with (p, dt, c) layout for linear DMA
        return np.ascontiguousarray(
            w[:, cs].reshape(ND, 128, 128).transpose(1, 0, 2).reshape(128, ND * 128)
        ).astype(bf)

    xT = [np.ascontiguousarray(x[b].T).astype(bf) for b in range(NB)]
    in_maps = []
    for c in range(NCORES):
        cs = slice(128 * c, 128 * (c + 1))
        in_maps.append(
            {
                "xT0": xT[0],
                "xT1": xT[1],
                "wq": rearr(wq, cs),
                "wk": rearr(wk, cs),
                "wv": rearr(wv, cs),
                "wo": np.ascontiguousarray(wo[cs, :]).astype(bf),
            }
        )
    return in_maps


def kernel(x, wq, wk, wv, wo, bo):
    from concourse.bass_utils import run_bass_kernel_spmd

    x = np.asarray(x, dtype=np.float32)
    wq = np.asarray(wq, dtype=np.float32)
    wk = np.asarray(wk, dtype=np.float32)
    wv = np.asarray(wv, dtype=np.float32)
    wo = np.asarray(wo, dtype=np.float32)
    bo = np.asarray(bo, dtype=np.float32)

    nc = _get_nc()
    in_maps = make_in_maps(x, wq, wk, wv, wo)
    try:
        res = run_bass_kernel_spmd(nc, in_maps, core_ids=list(range(NCORES))).results
    except Exception:
        res = run_bass_kernel_spmd(nc, in_maps, core_ids=list(range(NCORES))).results
    out = np.zeros((2, T, D), dtype=np.float32)
    for c in range(NCORES):
        for b in range(NB):
            out[b] += res[c][f"out{b}"].astype(np.float32)
    out += bo[None, None, :]
    return out
